# revision 30
# baseline (speedup 1.0000x reference)
"""IntrinsicRewardModule on 8 Trainium2 NeuronCores (Bass/Tile, SPMD).

Computation (reference semantics):
    r_raw[b] = mean_d (z_pred[b,d] - z_target[b,d])^2          # (B,)
    batch Welford merge (Chan) with incoming (count, mean, M2) scalars
    out = LAMBDA * (r_raw - new_mean) / (std + EPS)

Strategy: data-parallel over B across 8 cores (4096 rows each).
Per core: stream 32 tiles of [128 rows x 4096 cols] with contiguous
row-blocks (sequential HBM reads sustain ~345 GB/s vs ~320 strided),
DVE subtract + ACT Square-with-row-accumulate for per-row sums.
Per-shard (S, X) stats - X being the locally-centered M2 plus the
BL*mean^2 correction, so no catastrophic cancellation - are exchanged
with a 32-B-per-rank AllGather (an AllReduce costs ~2x the ring steps;
gathering the full r vector costs ~2x the payload latency) and merged
identically on every core via host-folded Chan coefficients.

The final row-tile is column-chunked so the post-stream pipeline drain
is ~3 us instead of ~8. A dummy warmup collective with the same shape
runs during the stream to absorb one-time collective setup costs.
Measured: ~449-452 us HW exec (baseline AllGather-r version: ~503 us);
rel err vs fp32 reference ~3.5e-5.
"""

import numpy as np

import concourse.bacc as bacc
import concourse.bass_isa as bass_isa
import concourse.mybir as mybir
import concourse.tile as tile
from concourse.bass_utils import run_bass_kernel_spmd

FP32 = mybir.dt.float32
ALU = mybir.AluOpType
ACT_FN = mybir.ActivationFunctionType

B, D = 32768, 4096
N_CORES = 8
BL = B // N_CORES          # rows per core (4096)
P = 128                    # SBUF partitions
T = BL // P                # row-tiles per core (32)
LAMBDA_INT = 0.01
EPS = 1e-8
NCH = 4                    # column chunks for the final row-tile
C = D // NCH

_nc_cache: dict = {}


def _build(is_small: bool, sub_mode: str = "dma_sub", warmup: bool = True):
    """Trace + compile the per-core Bass program.

    is_small: compile-time branch of the reference's `new_count < 2`
    (host knows new_count from the scalar inputs before compiling).
    sub_mode: 'dma_sub'  - z_target DMA with accum_op=subtract (HW rejects)
              'dma_add'  - z_target pre-negated on host, accum_op=add
                           (hangs the device on this runtime - do not use)
              'dve'      - separate load + DVE subtract (partition-major rows)
              'dve2'     - like dve, but z_target loads issue on the ACT
                           HWDGE ring (measured slower - ring interference)
              'dvec'     - dve with contiguous row-blocks per tile
              'dvei'     - dvec with z_pred|z_target host-interleaved per row:
                           one 4 MiB fully-contiguous DMA per tile
    """
    nc = bacc.Bacc(
        "TRN2", target_bir_lowering=False, debug=False, num_devices=N_CORES
    )
    interleaved = sub_mode == "dvei"
    if interleaved:
        zz = nc.dram_tensor("zz", [BL, 2 * D], FP32, kind="ExternalInput")
    else:
        zp = nc.dram_tensor("zp", [BL, D], FP32, kind="ExternalInput")
        zt = nc.dram_tensor("zt", [BL, D], FP32, kind="ExternalInput")
    # Host-precomputed per-partition-replicated Chan-merge coefficients:
    # [a0, a1, c0, c1, c2, inv_dc, pad*2] (see _run for the folding)
    params = nc.dram_tensor("params", [P, 8], FP32, kind="ExternalInput")
    out = nc.dram_tensor("out", [BL], FP32, kind="ExternalOutput")

    accum_alu = ALU.subtract if sub_mode == "dma_sub" else ALU.add

    if interleaved:
        # contiguous row-blocks of the host-interleaved [zp_row|zt_row]
        # array: one fully-sequential 4 MiB read per tile
        zz_v = zz.ap().rearrange("(t p) d -> t p d", p=P)
    elif sub_mode == "dvec":
        # contiguous layout: tile t = rows [t*128, (t+1)*128) — sequential
        # HBM reads; output needs a block transpose before the store
        zp_v = zp.ap().rearrange("(t p) d -> t p d", p=P)
        zt_v = zt.ap().rearrange("(t p) d -> t p d", p=P)
    else:
        # partition-major: row = p*T + t, output DMA contiguous per partition
        zp_v = zp.ap().rearrange("(p t) d -> t p d", p=P)
        zt_v = zt.ap().rearrange("(p t) d -> t p d", p=P)

    with tile.TileContext(nc) as tc:
        with (
            tc.tile_pool(name="pa", bufs=5) as pa,
            tc.tile_pool(name="pb", bufs=3) as pb,
            tc.tile_pool(name="stat", bufs=1) as ps,
            tc.tile_pool(name="dram", bufs=1, space="DRAM") as pdram,
        ):
            # rsum col t<T-1: full-tile row sums; cols T-1..T+2: chunks
            rsum = ps.tile([P, T + NCH - 1], FP32)

            # issue tile 0's bulk load before params/warmup so the stream
            # owns the DMA queue head
            if interleaved:
                td0 = pa.tile([P, 2 * D], FP32, tag="td")
                nc.sync.dma_start(td0[:], zz_v[0])
                tb0 = None
            else:
                td0 = pa.tile([P, D], FP32, tag="td")
                nc.sync.dma_start(td0[:], zp_v[0])
                tb0 = None
                if sub_mode.startswith("dve"):
                    tb0 = pb.tile([P, D], FP32, tag="tb")
                    zt_dma0 = (
                        nc.scalar.dma_start
                        if sub_mode == "dve2"
                        else nc.sync.dma_start
                    )
                    zt_dma0(tb0[:], zt_v[0])

            params_sb = ps.tile([P, 8], FP32)
            nc.sync.dma_start(params_sb[:], params.ap())

            # ---- warmup collective (absorbs one-time CC setup; same op
            # and shapes as the real one) ----
            if warmup:
                wdum_in = pdram.tile([8], FP32)
                wdum_out = pdram.tile([8 * N_CORES], FP32)
                wseed = ps.tile([1, 8], FP32)
                nc.vector.memset(wseed[:], 0.0)
                nc.sync.dma_start(
                    wdum_in[:].rearrange("(a b) -> a b", a=1), wseed[:]
                )
                nc.gpsimd.collective_compute(
                    "AllGather",
                    ALU.bypass,
                    replica_groups=[list(range(N_CORES))],
                    ins=[wdum_in.opt()],
                    outs=[wdum_out.opt()],
                )

            # ---- phase 1: stream z_pred/z_target, accumulate row sums ----
            zt_dma = nc.scalar.dma_start if sub_mode == "dve2" else nc.sync.dma_start
            for t in range(T - 1):
                if interleaved:
                    if t == 0:
                        td = td0
                    else:
                        td = pa.tile([P, 2 * D], FP32, tag="td")
                        nc.sync.dma_start(td[:], zz_v[t])
                    nc.vector.tensor_tensor(
                        td[:, :D], td[:, :D], td[:, D:], ALU.subtract
                    )
                    nc.scalar.activation(
                        td[:, :D], td[:, :D], ACT_FN.Square,
                        accum_out=rsum[:, t : t + 1],
                    )
                    continue
                if t == 0:
                    td, tb = td0, tb0
                else:
                    td = pa.tile([P, D], FP32, tag="td")
                    nc.sync.dma_start(td[:], zp_v[t])
                    tb = None
                    if sub_mode.startswith("dve"):
                        tb = pb.tile([P, D], FP32, tag="tb")
                        zt_dma(tb[:], zt_v[t])
                if sub_mode.startswith("dve"):
                    nc.vector.tensor_tensor(td[:], td[:], tb[:], ALU.subtract)
                else:
                    nc.gpsimd.dma_start(
                        td[:], zt_v[t], accum_op=accum_alu,
                        max_dma_last_dim=2048,
                    )
                nc.scalar.activation(
                    td[:], td[:], ACT_FN.Square, accum_out=rsum[:, t : t + 1]
                )
            # final row-tile: column chunks to shorten the serial drain
            t = T - 1
            for k in range(NCH):
                cs = slice(k * C, (k + 1) * C)
                tdk = pb.tile([P, C], FP32, tag="tdk")
                if interleaved:
                    tbk = pb.tile([P, C], FP32, tag="tbk")
                    nc.sync.dma_start(tdk[:], zz_v[t][:, cs])
                    nc.sync.dma_start(
                        tbk[:], zz_v[t][:, D + k * C : D + (k + 1) * C]
                    )
                    nc.vector.tensor_tensor(tdk[:], tdk[:], tbk[:], ALU.subtract)
                elif sub_mode.startswith("dve"):
                    tbk = pb.tile([P, C], FP32, tag="tbk")
                    nc.sync.dma_start(tdk[:], zp_v[t][:, cs])
                    zt_dma(tbk[:], zt_v[t][:, cs])
                    nc.vector.tensor_tensor(tdk[:], tdk[:], tbk[:], ALU.subtract)
                else:
                    nc.sync.dma_start(tdk[:], zp_v[t][:, cs])
                    nc.gpsimd.dma_start(
                        tdk[:], zt_v[t][:, cs], accum_op=accum_alu,
                        max_dma_last_dim=2048,
                    )
                nc.scalar.activation(
                    tdk[:], tdk[:], ACT_FN.Square,
                    accum_out=rsum[:, T - 1 + k : T + k],
                )

            # ---- phase 2: local shard stats, all in rsum units (D*r) ----
            # S'_l = sum(rsum); X'_l = q' + BL*(D*mean_l)^2 where q' is the
            # locally-centered sum of squares (no catastrophic cancellation).
            # Host coefficients absorb the 1/D scalings.
            sp = ps.tile([P, 1], FP32)
            nc.vector.reduce_sum(sp[:], rsum[:], axis=mybir.AxisListType.X)
            lastsum = ps.tile([P, 1], FP32)
            nc.vector.reduce_sum(
                lastsum[:], rsum[:, T - 1 :], axis=mybir.AxisListType.X
            )
            pack = ps.tile([P, 8], FP32)
            nc.vector.memset(pack[:], 0.0)
            nc.gpsimd.partition_all_reduce(
                pack[:, 0:1], sp[:], channels=P,
                reduce_op=bass_isa.ReduceOp.add,
            )
            mean_r = ps.tile([P, 1], FP32)
            nc.vector.tensor_scalar_mul(mean_r[:], pack[:, 0:1], 1.0 / BL)
            cent = ps.tile([P, T], FP32)
            nc.vector.tensor_scalar(
                cent[:, : T - 1], rsum[:, : T - 1], mean_r[:], None,
                ALU.subtract,
            )
            nc.vector.tensor_scalar(
                cent[:, T - 1 : T], lastsum[:], mean_r[:], None, ALU.subtract
            )
            qp = ps.tile([P, 1], FP32)
            nc.scalar.activation(
                cent[:], cent[:], ACT_FN.Square, accum_out=qp[:]
            )
            ql = ps.tile([P, 1], FP32)
            nc.gpsimd.partition_all_reduce(
                ql[:], qp[:], channels=P, reduce_op=bass_isa.ReduceOp.add
            )
            m2m = ps.tile([P, 1], FP32)
            nc.vector.tensor_tensor(m2m[:], mean_r[:], mean_r[:], ALU.mult)
            nc.vector.scalar_tensor_tensor(
                pack[:, 1:2], m2m[:], float(BL), ql[:],
                op0=ALU.mult, op1=ALU.add,
            )

            # ---- 32B-per-rank AllGather of (S_l, X_l) across the 8 cores ----
            stats_in = pdram.tile([8], FP32)
            stats_out = pdram.tile([8 * N_CORES], FP32)
            nc.sync.dma_start(
                stats_in[:].rearrange("(a b) -> a b", a=1), pack[0:1, :]
            )
            nc.gpsimd.collective_compute(
                "AllGather",
                ALU.bypass,
                replica_groups=[list(range(N_CORES))],
                ins=[stats_in.opt()],
                outs=[stats_out.opt()],
            )
            # one rank per partition; partition_all_reduce sums the 8 ranks
            g8 = ps.tile([N_CORES, 8], FP32)
            nc.sync.dma_start(
                g8[:], stats_out[:].rearrange("(r e) -> r e", r=N_CORES)
            )
            g8s = ps.tile([N_CORES, 8], FP32)
            nc.gpsimd.partition_all_reduce(
                g8s[:], g8[:], channels=N_CORES,
                reduce_op=bass_isa.ReduceOp.add,
            )
            gb = ps.tile([P, 8], FP32)
            nc.gpsimd.partition_broadcast(gb[:], g8s[0:1, :], channels=P)

            # ---- global stats + Chan merge (host-folded coefficients) ----
            # u = S'/(B*D) = batch mean of r ; new_mean = a1*u + a0
            # new_M2 = X'/D^2 + (c2*u + c1)*u + c0
            # scale = LAMBDA / sqrt(inv_dc * new_M2)
            a0 = params_sb[:, 0:1]
            a1 = params_sb[:, 1:2]
            c0 = params_sb[:, 2:3]
            c1 = params_sb[:, 3:4]
            c2 = params_sb[:, 4:5]
            inv_dc = params_sb[:, 5:6]

            S = gb[:, 0:1]
            X = gb[:, 1:2]
            u = ps.tile([P, 1], FP32)
            nc.vector.tensor_scalar_mul(u[:], S, 1.0 / (float(B) * D))
            new_mean = ps.tile([P, 1], FP32)
            nc.vector.tensor_scalar(
                new_mean[:], u[:], a1, a0, ALU.mult, ALU.add
            )
            scale_pp = ps.tile([P, 1], FP32)
            if is_small:
                # reference: std = 1.0 when new_count < 2
                nc.vector.memset(scale_pp[:], LAMBDA_INT / (1.0 + 2.0 * EPS))
            else:
                q1 = ps.tile([P, 1], FP32)
                nc.vector.tensor_scalar(
                    q1[:], u[:], c2, c1, ALU.mult, ALU.add
                )
                q2 = ps.tile([P, 1], FP32)
                nc.vector.tensor_tensor(q2[:], q1[:], u[:], ALU.mult)
                xs = ps.tile([P, 1], FP32)
                nc.vector.tensor_scalar_mul(xs[:], X, 1.0 / (float(D) * D))
                new_m2 = ps.tile([P, 1], FP32)
                nc.vector.tensor_scalar(
                    new_m2[:], q2[:], c0, xs[:], ALU.add, ALU.add
                )
                # var = new_m2 * inv_dc ; scale = LAMBDA / sqrt(var)
                # (the reference's +EPS on std is ~5e-7 relative; folded away)
                var = ps.tile([P, 1], FP32)
                nc.vector.tensor_tensor(var[:], new_m2[:], inv_dc, ALU.mult)
                std = ps.tile([P, 1], FP32)
                nc.scalar.activation(std[:], var[:], ACT_FN.Sqrt)
                inv = ps.tile([P, 1], FP32)
                nc.vector.reciprocal(inv[:], std[:])
                nc.vector.tensor_scalar_mul(scale_pp[:], inv[:], LAMBDA_INT)

            # out = (rsum - D*new_mean) * (scale/D), written per-column-group
            dnm = ps.tile([P, 1], FP32)
            nc.vector.tensor_scalar_mul(dnm[:], new_mean[:], float(D))
            sD = ps.tile([P, 1], FP32)
            nc.vector.tensor_scalar_mul(sD[:], scale_pp[:], 1.0 / D)
            out_sb = ps.tile([P, T], FP32)
            nc.vector.tensor_scalar(
                out_sb[:, : T - 1], rsum[:, : T - 1], dnm[:], sD[:],
                ALU.subtract, ALU.mult,
            )
            nc.vector.tensor_scalar(
                out_sb[:, T - 1 : T], lastsum[:], dnm[:], sD[:],
                ALU.subtract, ALU.mult,
            )
            if sub_mode in ("dvec", "dvei"):
                # out_sb[p, t] holds row t*128+p. DVE transposes each 32x32
                # block in place; the DRAM AP reorders the blocks so every
                # partition line is a contiguous 128 B run.
                out_t = ps.tile([P, T], FP32)
                nc.vector.transpose(out_t[:], out_sb[:])
                out_2d = out.ap().rearrange("(a c) -> a c", a=32)
                for i in range(4):
                    nc.sync.dma_start(
                        out_2d[:, 32 * i : 32 * (i + 1)],
                        out_t[32 * i : 32 * (i + 1), :],
                    )
            else:
                nc.sync.dma_start(
                    out.ap().rearrange("(p t) -> p t", p=P), out_sb[:]
                )

    nc.compile()
    return nc


import os

SUB_MODE = os.environ.get("K_SUB_MODE", "dvec")
WARMUP = os.environ.get("K_WARMUP", "1") == "1"


def _get_nc(is_small: bool):
    key = (is_small, SUB_MODE, WARMUP)
    if key not in _nc_cache:
        _nc_cache[key] = _build(is_small, SUB_MODE, WARMUP)
    return _nc_cache[key]


def _run(z_pred, z_target, count, mean, M2, trace=False):
    z_pred = np.ascontiguousarray(np.asarray(z_pred, dtype=np.float32))
    z_target = np.asarray(z_target, dtype=np.float32)
    if SUB_MODE == "dma_add":
        z_target = np.ascontiguousarray(-z_target)
    else:
        z_target = np.ascontiguousarray(z_target)
    assert z_pred.shape == (B, D) and z_target.shape == (B, D)

    count_f = float(np.asarray(count))
    mean_f = float(np.asarray(mean))
    m2_f = float(np.asarray(M2))

    n = float(B)
    new_count = count_f + n
    n_over = n / new_count
    chan_c = count_f * n / new_count
    inv_dc = 1.0 / max(new_count - 1.0, 1.0)
    is_small = new_count < 2.0

    # new_mean = a1*u + a0 where u = S/B (batch mean)
    a0 = mean_f * (1.0 - n_over)
    a1 = n_over
    # new_M2 = X + c2*u^2 + c1*u + c0
    c2 = chan_c - n
    c1 = -2.0 * chan_c * mean_f
    c0 = m2_f + chan_c * mean_f * mean_f
    prow = np.array(
        [[a0, a1, c0, c1, c2, inv_dc, 0.0, 0.0]], dtype=np.float32
    )
    params = np.ascontiguousarray(np.tile(prow, (P, 1)))

    nc = _get_nc(is_small)
    if SUB_MODE == "dvei":
        in_maps = [
            {
                "zz": np.ascontiguousarray(
                    np.concatenate(
                        [
                            z_pred[c * BL : (c + 1) * BL],
                            z_target[c * BL : (c + 1) * BL],
                        ],
                        axis=1,
                    )
                ),
                "params": params,
            }
            for c in range(N_CORES)
        ]
    else:
        in_maps = [
            {
                "zp": z_pred[c * BL : (c + 1) * BL],
                "zt": z_target[c * BL : (c + 1) * BL],
                "params": params,
            }
            for c in range(N_CORES)
        ]
    res = run_bass_kernel_spmd(
        nc, in_maps, core_ids=list(range(N_CORES)), trace=trace
    )
    out = np.concatenate([res.results[c]["out"] for c in range(N_CORES)])
    return out.astype(np.float32), res


def kernel(z_pred, z_target, count, mean, M2):
    out, _ = _run(z_pred, z_target, count, mean, M2, trace=False)
    return out


# revision 33
# speedup vs baseline: 1.0301x; 1.0301x over previous
"""IntrinsicRewardModule on 8 Trainium2 NeuronCores (Bass/Tile, SPMD).

Computation (reference semantics):
    r_raw[b] = mean_d (z_pred[b,d] - z_target[b,d])^2          # (B,)
    batch Welford merge (Chan) with incoming (count, mean, M2) scalars
    out = LAMBDA * (r_raw - new_mean) / (std + EPS)

Strategy: data-parallel over B across 8 cores (4096 rows each).
Per core: stream 32 tiles of [128 rows x 4096 cols] with contiguous
row-blocks (sequential HBM reads sustain ~345 GB/s vs ~320 strided),
DVE subtract + ACT Square-with-row-accumulate for per-row sums.
Per-shard (S, X) stats - X being the locally-centered M2 plus the
BL*mean^2 correction, so no catastrophic cancellation - are exchanged
with a 32-B-per-rank AllGather (an AllReduce costs ~2x the ring steps;
gathering the full r vector costs ~2x the payload latency) and merged
identically on every core via host-folded Chan coefficients.

The final row-tile is column-chunked so the post-stream pipeline drain
is ~3 us instead of ~8. A dummy warmup collective with the same shape
runs during the stream to absorb one-time collective setup costs.
Measured: ~449-452 us HW exec (baseline AllGather-r version: ~503 us);
rel err vs fp32 reference ~3.5e-5.
"""

import numpy as np

import concourse.bacc as bacc
import concourse.bass_isa as bass_isa
import concourse.mybir as mybir
import concourse.tile as tile
from concourse.bass_utils import run_bass_kernel_spmd

FP32 = mybir.dt.float32
ALU = mybir.AluOpType
ACT_FN = mybir.ActivationFunctionType

B, D = 32768, 4096
N_CORES = 8
BL = B // N_CORES          # rows per core (4096)
P = 128                    # SBUF partitions
T = BL // P                # row-tiles per core (32)
LAMBDA_INT = 0.01
EPS = 1e-8
NCH = 4                    # column chunks for the final row-tile
C = D // NCH

_nc_cache: dict = {}


def _build(is_small: bool, sub_mode: str = "dma_sub", warmup: bool = True):
    """Trace + compile the per-core Bass program.

    is_small: compile-time branch of the reference's `new_count < 2`
    (host knows new_count from the scalar inputs before compiling).
    sub_mode: 'dma_sub'  - z_target DMA with accum_op=subtract (HW rejects)
              'dma_add'  - z_target pre-negated on host, accum_op=add
                           (hangs the device on this runtime - do not use)
              'dve'      - separate load + DVE subtract (partition-major rows)
              'dve2'     - like dve, but z_target loads issue on the ACT
                           HWDGE ring (measured slower - ring interference)
              'dvec'     - dve with contiguous row-blocks per tile
              'dvei'     - dvec with z_pred|z_target host-interleaved per row:
                           one 4 MiB fully-contiguous DMA per tile
    """
    nc = bacc.Bacc(
        "TRN2", target_bir_lowering=False, debug=False, num_devices=N_CORES
    )
    interleaved = sub_mode == "dvei"
    if interleaved:
        zz = nc.dram_tensor("zz", [BL, 2 * D], FP32, kind="ExternalInput")
    else:
        zp = nc.dram_tensor("zp", [BL, D], FP32, kind="ExternalInput")
        zt = nc.dram_tensor("zt", [BL, D], FP32, kind="ExternalInput")
    # Host-precomputed per-partition-replicated Chan-merge coefficients:
    # [a0, a1, c0, c1, c2, inv_dc, pad*2] (see _run for the folding)
    params = nc.dram_tensor("params", [P, 8], FP32, kind="ExternalInput")
    out = nc.dram_tensor("out", [BL], FP32, kind="ExternalOutput")

    accum_alu = ALU.subtract if sub_mode == "dma_sub" else ALU.add

    if interleaved:
        # contiguous row-blocks of the host-interleaved [zp_row|zt_row]
        # array: one fully-sequential 4 MiB read per tile
        zz_v = zz.ap().rearrange("(t p) d -> t p d", p=P)
    elif sub_mode == "dvec":
        # contiguous layout: tile t = rows [t*128, (t+1)*128) — sequential
        # HBM reads; output needs a block transpose before the store
        zp_v = zp.ap().rearrange("(t p) d -> t p d", p=P)
        zt_v = zt.ap().rearrange("(t p) d -> t p d", p=P)
    else:
        # partition-major: row = p*T + t, output DMA contiguous per partition
        zp_v = zp.ap().rearrange("(p t) d -> t p d", p=P)
        zt_v = zt.ap().rearrange("(p t) d -> t p d", p=P)

    with tile.TileContext(nc) as tc:
        with (
            tc.tile_pool(name="pa", bufs=5) as pa,
            tc.tile_pool(name="pb", bufs=3) as pb,
            tc.tile_pool(name="stat", bufs=1) as ps,
            tc.tile_pool(name="dram", bufs=1, space="DRAM") as pdram,
        ):
            # rsum col t<T-1: full-tile row sums; cols T-1..T+2: chunks
            rsum = ps.tile([P, T + NCH - 1], FP32)

            # issue tile 0's bulk load before params/warmup so the stream
            # owns the DMA queue head
            if interleaved:
                td0 = pa.tile([P, 2 * D], FP32, tag="td")
                nc.sync.dma_start(td0[:], zz_v[0])
                tb0 = None
            else:
                td0 = pa.tile([P, D], FP32, tag="td")
                nc.sync.dma_start(td0[:], zp_v[0])
                tb0 = None
                if sub_mode.startswith("dve"):
                    tb0 = pb.tile([P, D], FP32, tag="tb")
                    zt_dma0 = (
                        nc.scalar.dma_start
                        if sub_mode == "dve2"
                        else nc.sync.dma_start
                    )
                    zt_dma0(tb0[:], zt_v[0])

            params_sb = ps.tile([P, 8], FP32)
            nc.sync.dma_start(params_sb[:], params.ap())

            # ---- warmup collective (absorbs one-time CC setup; same op
            # and shapes as the real one) ----
            if warmup:
                wdum_in = pdram.tile([8], FP32)
                wdum_out = pdram.tile([8 * N_CORES], FP32)
                wseed = ps.tile([1, 8], FP32)
                nc.vector.memset(wseed[:], 0.0)
                nc.sync.dma_start(
                    wdum_in[:].rearrange("(a b) -> a b", a=1), wseed[:]
                )
                nc.gpsimd.collective_compute(
                    "AllGather",
                    ALU.bypass,
                    replica_groups=[list(range(N_CORES))],
                    ins=[wdum_in.opt()],
                    outs=[wdum_out.opt()],
                )

            # ---- phase 1: stream z_pred/z_target, accumulate row sums ----
            zt_dma = nc.scalar.dma_start if sub_mode == "dve2" else nc.sync.dma_start
            for t in range(T - 1):
                if interleaved:
                    if t == 0:
                        td = td0
                    else:
                        td = pa.tile([P, 2 * D], FP32, tag="td")
                        nc.sync.dma_start(td[:], zz_v[t])
                    nc.vector.tensor_tensor(
                        td[:, :D], td[:, :D], td[:, D:], ALU.subtract
                    )
                    nc.scalar.activation(
                        td[:, :D], td[:, :D], ACT_FN.Square,
                        accum_out=rsum[:, t : t + 1],
                    )
                    continue
                if t == 0:
                    td, tb = td0, tb0
                else:
                    td = pa.tile([P, D], FP32, tag="td")
                    nc.sync.dma_start(td[:], zp_v[t])
                    tb = None
                    if sub_mode.startswith("dve"):
                        tb = pb.tile([P, D], FP32, tag="tb")
                        zt_dma(tb[:], zt_v[t])
                if sub_mode.startswith("dve"):
                    nc.vector.tensor_tensor(td[:], td[:], tb[:], ALU.subtract)
                else:
                    nc.gpsimd.dma_start(
                        td[:], zt_v[t], accum_op=accum_alu,
                        max_dma_last_dim=2048,
                    )
                nc.scalar.activation(
                    td[:], td[:], ACT_FN.Square, accum_out=rsum[:, t : t + 1]
                )
            # final row-tile: column chunks to shorten the serial drain
            t = T - 1
            for k in range(NCH):
                cs = slice(k * C, (k + 1) * C)
                tdk = pb.tile([P, C], FP32, tag="tdk")
                if interleaved:
                    tbk = pb.tile([P, C], FP32, tag="tbk")
                    nc.sync.dma_start(tdk[:], zz_v[t][:, cs])
                    nc.sync.dma_start(
                        tbk[:], zz_v[t][:, D + k * C : D + (k + 1) * C]
                    )
                    nc.vector.tensor_tensor(tdk[:], tdk[:], tbk[:], ALU.subtract)
                elif sub_mode.startswith("dve"):
                    tbk = pb.tile([P, C], FP32, tag="tbk")
                    nc.sync.dma_start(tdk[:], zp_v[t][:, cs])
                    zt_dma(tbk[:], zt_v[t][:, cs])
                    nc.vector.tensor_tensor(tdk[:], tdk[:], tbk[:], ALU.subtract)
                else:
                    nc.sync.dma_start(tdk[:], zp_v[t][:, cs])
                    nc.gpsimd.dma_start(
                        tdk[:], zt_v[t][:, cs], accum_op=accum_alu,
                        max_dma_last_dim=2048,
                    )
                nc.scalar.activation(
                    tdk[:], tdk[:], ACT_FN.Square,
                    accum_out=rsum[:, T - 1 + k : T + k],
                )

            # ---- phase 2: local shard stats, all in rsum units (D*r) ----
            # S'_l = sum(rsum); X'_l = q' + BL*(D*mean_l)^2 where q' is the
            # locally-centered sum of squares (no catastrophic cancellation).
            # Host coefficients absorb the 1/D scalings. The T-1 full-tile
            # columns reduce during the chunk phase; only the chunk columns
            # and one add sit on the post-stream critical path.
            sp_part = ps.tile([P, 1], FP32)
            nc.vector.reduce_sum(
                sp_part[:], rsum[:, : T - 1], axis=mybir.AxisListType.X
            )
            lastsum = ps.tile([P, 1], FP32)
            nc.vector.reduce_sum(
                lastsum[:], rsum[:, T - 1 :], axis=mybir.AxisListType.X
            )
            sp = ps.tile([P, 1], FP32)
            nc.vector.tensor_tensor(sp[:], sp_part[:], lastsum[:], ALU.add)
            pack = ps.tile([P, 8], FP32)
            nc.vector.memset(pack[:], 0.0)
            nc.gpsimd.partition_all_reduce(
                pack[:, 0:1], sp[:], channels=P,
                reduce_op=bass_isa.ReduceOp.add,
            )
            mean_r = ps.tile([P, 1], FP32)
            nc.vector.tensor_scalar_mul(mean_r[:], pack[:, 0:1], 1.0 / BL)
            cent = ps.tile([P, T], FP32)
            nc.vector.tensor_scalar(
                cent[:, : T - 1], rsum[:, : T - 1], mean_r[:], None,
                ALU.subtract,
            )
            nc.vector.tensor_scalar(
                cent[:, T - 1 : T], lastsum[:], mean_r[:], None, ALU.subtract
            )
            qp = ps.tile([P, 1], FP32)
            nc.scalar.activation(
                cent[:], cent[:], ACT_FN.Square, accum_out=qp[:]
            )
            ql = ps.tile([P, 1], FP32)
            nc.gpsimd.partition_all_reduce(
                ql[:], qp[:], channels=P, reduce_op=bass_isa.ReduceOp.add
            )
            m2m = ps.tile([P, 1], FP32)
            nc.vector.tensor_tensor(m2m[:], mean_r[:], mean_r[:], ALU.mult)
            nc.vector.scalar_tensor_tensor(
                pack[:, 1:2], m2m[:], float(BL), ql[:],
                op0=ALU.mult, op1=ALU.add,
            )

            # ---- 32B-per-rank AllGather of (S_l, X_l) across the 8 cores ----
            stats_in = pdram.tile([8], FP32)
            stats_out = pdram.tile([8 * N_CORES], FP32)
            nc.sync.dma_start(
                stats_in[:].rearrange("(a b) -> a b", a=1), pack[0:1, :]
            )
            nc.gpsimd.collective_compute(
                "AllGather",
                ALU.bypass,
                replica_groups=[list(range(N_CORES))],
                ins=[stats_in.opt()],
                outs=[stats_out.opt()],
            )
            # one rank per partition; partition_all_reduce sums the 8 ranks
            g8 = ps.tile([N_CORES, 8], FP32)
            nc.sync.dma_start(
                g8[:], stats_out[:].rearrange("(r e) -> r e", r=N_CORES)
            )
            g8s = ps.tile([N_CORES, 8], FP32)
            nc.gpsimd.partition_all_reduce(
                g8s[:], g8[:], channels=N_CORES,
                reduce_op=bass_isa.ReduceOp.add,
            )
            gb = ps.tile([P, 8], FP32)
            nc.gpsimd.partition_broadcast(gb[:], g8s[0:1, :], channels=P)

            # ---- global stats + Chan merge (host-folded coefficients) ----
            # dnm = D*new_mean = k1*S' + a0d  (k1 = a1/B, a0d = D*a0)
            # u = S'/(B*D); new_M2 = X'/D^2 + (c2*u + c1)*u + c0
            # sD = scale/D = (LAMBDA/D) / sqrt(inv_dc * new_M2)
            k1 = params_sb[:, 0:1]
            a0d = params_sb[:, 1:2]
            c0 = params_sb[:, 2:3]
            c1 = params_sb[:, 3:4]
            c2 = params_sb[:, 4:5]
            inv_dc = params_sb[:, 5:6]

            S = gb[:, 0:1]
            X = gb[:, 1:2]
            dnm = ps.tile([P, 1], FP32)
            nc.vector.tensor_scalar(dnm[:], S, k1, a0d, ALU.mult, ALU.add)
            sD = ps.tile([P, 1], FP32)
            if is_small:
                # reference: std = 1.0 when new_count < 2
                nc.vector.memset(
                    sD[:], LAMBDA_INT / (D * (1.0 + 2.0 * EPS))
                )
            else:
                u = ps.tile([P, 1], FP32)
                nc.vector.tensor_scalar_mul(u[:], S, 1.0 / (float(B) * D))
                q1 = ps.tile([P, 1], FP32)
                nc.vector.tensor_scalar(
                    q1[:], u[:], c2, c1, ALU.mult, ALU.add
                )
                q2 = ps.tile([P, 1], FP32)
                nc.vector.tensor_tensor(q2[:], q1[:], u[:], ALU.mult)
                xs = ps.tile([P, 1], FP32)
                nc.vector.tensor_scalar_mul(xs[:], X, 1.0 / (float(D) * D))
                new_m2 = ps.tile([P, 1], FP32)
                nc.vector.tensor_scalar(
                    new_m2[:], q2[:], c0, xs[:], ALU.add, ALU.add
                )
                # var = new_m2 * inv_dc ; sD = (LAMBDA/D) / sqrt(var)
                # (the reference's +EPS on std is ~5e-7 relative; folded away)
                var = ps.tile([P, 1], FP32)
                nc.vector.tensor_tensor(var[:], new_m2[:], inv_dc, ALU.mult)
                std = ps.tile([P, 1], FP32)
                nc.scalar.activation(std[:], var[:], ACT_FN.Sqrt)
                inv = ps.tile([P, 1], FP32)
                nc.vector.reciprocal(inv[:], std[:])
                nc.vector.tensor_scalar_mul(sD[:], inv[:], LAMBDA_INT / D)
            out_sb = ps.tile([P, T], FP32)
            nc.vector.tensor_scalar(
                out_sb[:, : T - 1], rsum[:, : T - 1], dnm[:], sD[:],
                ALU.subtract, ALU.mult,
            )
            nc.vector.tensor_scalar(
                out_sb[:, T - 1 : T], lastsum[:], dnm[:], sD[:],
                ALU.subtract, ALU.mult,
            )
            if sub_mode in ("dvec", "dvei"):
                # out_sb[p, t] holds row t*128+p. DVE transposes each 32x32
                # block in place; the DRAM AP reorders the blocks so every
                # partition line is a contiguous 128 B run.
                out_t = ps.tile([P, T], FP32)
                nc.vector.transpose(out_t[:], out_sb[:])
                out_2d = out.ap().rearrange("(a c) -> a c", a=32)
                for i in range(4):
                    nc.sync.dma_start(
                        out_2d[:, 32 * i : 32 * (i + 1)],
                        out_t[32 * i : 32 * (i + 1), :],
                    )
            else:
                nc.sync.dma_start(
                    out.ap().rearrange("(p t) -> p t", p=P), out_sb[:]
                )

    nc.compile()
    return nc


import os

SUB_MODE = os.environ.get("K_SUB_MODE", "dvec")
WARMUP = os.environ.get("K_WARMUP", "1") == "1"


def _get_nc(is_small: bool):
    key = (is_small, SUB_MODE, WARMUP)
    if key not in _nc_cache:
        _nc_cache[key] = _build(is_small, SUB_MODE, WARMUP)
    return _nc_cache[key]


def _run(z_pred, z_target, count, mean, M2, trace=False):
    z_pred = np.ascontiguousarray(np.asarray(z_pred, dtype=np.float32))
    z_target = np.asarray(z_target, dtype=np.float32)
    if SUB_MODE == "dma_add":
        z_target = np.ascontiguousarray(-z_target)
    else:
        z_target = np.ascontiguousarray(z_target)
    assert z_pred.shape == (B, D) and z_target.shape == (B, D)

    count_f = float(np.asarray(count))
    mean_f = float(np.asarray(mean))
    m2_f = float(np.asarray(M2))

    n = float(B)
    new_count = count_f + n
    n_over = n / new_count
    chan_c = count_f * n / new_count
    inv_dc = 1.0 / max(new_count - 1.0, 1.0)
    is_small = new_count < 2.0

    # new_mean = a1*u + a0 where u = S/B (batch mean); the kernel consumes
    # k1 = a1/B and a0d = D*a0 so D*new_mean comes from S' in one op
    a0 = mean_f * (1.0 - n_over)
    a1 = n_over
    k1 = a1 / n
    a0d = float(D) * a0
    # new_M2 = X + c2*u^2 + c1*u + c0
    c2 = chan_c - n
    c1 = -2.0 * chan_c * mean_f
    c0 = m2_f + chan_c * mean_f * mean_f
    prow = np.array(
        [[k1, a0d, c0, c1, c2, inv_dc, 0.0, 0.0]], dtype=np.float32
    )
    params = np.ascontiguousarray(np.tile(prow, (P, 1)))

    nc = _get_nc(is_small)
    if SUB_MODE == "dvei":
        in_maps = [
            {
                "zz": np.ascontiguousarray(
                    np.concatenate(
                        [
                            z_pred[c * BL : (c + 1) * BL],
                            z_target[c * BL : (c + 1) * BL],
                        ],
                        axis=1,
                    )
                ),
                "params": params,
            }
            for c in range(N_CORES)
        ]
    else:
        in_maps = [
            {
                "zp": z_pred[c * BL : (c + 1) * BL],
                "zt": z_target[c * BL : (c + 1) * BL],
                "params": params,
            }
            for c in range(N_CORES)
        ]
    res = run_bass_kernel_spmd(
        nc, in_maps, core_ids=list(range(N_CORES)), trace=trace
    )
    out = np.concatenate([res.results[c]["out"] for c in range(N_CORES)])
    return out.astype(np.float32), res


def kernel(z_pred, z_target, count, mean, M2):
    out, _ = _run(z_pred, z_target, count, mean, M2, trace=False)
    return out
